# revision 1
# baseline (speedup 1.0000x reference)
"""Trainium2 Bass kernel for PVT-style cross-batch sparse attention.

Reference computation (per batch element b, with partner p = (b+4)%8):
    q  = x[b] @ Wq.T                                  [4096, 128]
    xr = conv_stride4(x[p]) + sr_b  -> layernorm      [256, 128]
    kv = xr @ Wkv.T ; k, v heads (2 heads, hd=64)
    out = softmax(q k^T / sqrt(hd)) v ; proj Wp + bp  [4096, 128]

Sharding: one batch element per NeuronCore (8 cores). Each core gets its
own x (for q) and its partner's x (for k/v). No collectives.

Dataflow is channel-transposed ([C, T] layout):
  - x arrives HOST-pre-transposed (and bf16-cast) inside a single input
    blob per core, so the kernel needs no on-chip transposes and exactly
    one load DMA (keeps the tail drain under the sync-wait budget).
  - scores are computed transposed ([keys, q]); the softmax key-sum runs
    on the TensorEngine as a column-packed ones-matmul next to the A@V
    matmul (both heads concurrent in disjoint PE column/row groups).
  - matmuls run in bf16 (1 cycle/row vs 4 for fp32); scale and weight
    transposes are folded host-side; the output bias bp is added on host.
  - the hardware allows only ONE semaphore wait per engine instruction
    (plus its own update) and DMAs carry a queue-ring update, so the
    structure keeps every instruction's dependencies on a single
    not-yet-observed semaphore: single-producer tiles, dedicated psum
    tag chains, dummy "touch" ops to advance engine clocks, and output
    stores that are each the first DMA on their hardware queue.
"""

import numpy as np
import ml_dtypes

import concourse.bass as bass
from bass_rust import add_dep_helper
import concourse.tile as tile
from concourse import mybir
from concourse.bass_utils import run_bass_kernel_spmd


# ---------------------------------------------------------------------------
# The tail drain TileContext emits waits on every processor's final tick in
# ONE instruction, which exceeds this toolchain's per-instruction sync-wait
# budget. Split it: emit one single-wait drain per active proc first (the
# wait-clock elision then leaves the final drain with nothing to wait on).
from bass_rust import ScopedClock, VectorClock
from concourse.tile_scheduler import N_PROCS


def _split_drain_and_barrier(self, tick_clock, wait_clock):
    full = tick_clock.global_clock
    for p in range(N_PROCS):
        t = full[p]
        if t <= 0:
            continue
        ticks = [0] * N_PROCS
        ticks[p] = t
        d = self.nc.sync.drain()
        wait_clock.add_sem_waits(d.ins, ScopedClock({None: VectorClock(ticks)}))
    # the per-proc drains above run sequentially on the SP sequencer, so by
    # the time the last one retires every proc has reached its final tick -
    # the closing drain needs no waits of its own
    self.nc.sync.drain()

    self.nc.all_engine_barrier()
    assert self.sems is not None
    popped = self.nc._tile_sem_poison_stack.pop()
    assert popped is self._sem_poison
    self.nc.clear_and_free_semaphores(list(self.sems.allocated().values()))
    self.nc.all_engine_barrier()


tile.TileContext._drain_and_barrier = _split_drain_and_barrier

BF16 = mybir.dt.bfloat16
F32 = mybir.dt.float32

B, T, C = 8, 4096, 128
NH, HD = 2, 64
SR = 4
H = W = 64
OH = OW = 16
NK = OH * OW          # 256 reduced tokens
SCALE = HD ** -0.5
SC = 1024             # q chunk width for attention
NSC = T // SC

# blob column offsets (bf16 columns)
O_XQ = 0
O_XKV = O_XQ + T
O_WQ = O_XKV + T
O_WKV = O_WQ + C
O_WP = O_WKV + 2 * C
O_SRW = O_WP + C
O_ONES = O_SRW + 16 * C
O_INV = O_ONES + C    # 8 bf16 cols, col 0 = 1/128
O_VECS = O_INV + 8
NBLOB = O_VECS + 8    # 4 f32 (srb, lnw, lnb, eps) bit-cast to 8 bf16


def build_nc(out_bf16: bool = True, niter: int = 1,
             store_last_only: bool = False):
    nc = bass.Bass()

    blob = nc.declare_dram_parameter("blob", [C, NBLOB], BF16, isOutput=False)
    out_dt = BF16 if out_bf16 else F32
    out = nc.declare_dram_parameter("out", [T, C], out_dt, isOutput=True)

    with tile.TileContext(nc) as tc:
        const = tc.alloc_tile_pool(name="const", bufs=1)
        work = tc.alloc_tile_pool(name="work", bufs=2)
        # PSUM tags (8 banks):
        #   "big"  bufs=1 x [128,2048]f32 (4 banks): score tiles
        #   "b512" bufs=2 x [128,512]f32  (2 banks): qp, conv, av/dn
        #   "misc" bufs=2 x [128,512]f32  (2 banks): LN stats/bcast, kv, v, pj
        psum = tc.alloc_tile_pool(name="psum", bufs=1, space="PSUM")

        blob_sb = const.tile([128, NBLOB], BF16)
        # split into 3 DMAs on separate queues so compute can start as
        # soon as its slice lands (they share the full DMA bus)
        # three issuing engines so descriptor generation runs in parallel
        nc.sync.dma_start(out=blob_sb[:, O_XKV:O_XKV + T // 2],
                          in_=blob[:, O_XKV:O_XKV + T // 2])
        nc.sync.dma_start(out=blob_sb[:, O_XKV + T // 2:O_XKV + T],
                          in_=blob[:, O_XKV + T // 2:O_XKV + T])
        nc.scalar.dma_start(out=blob_sb[:, O_WQ:NBLOB], in_=blob[:, O_WQ:NBLOB])
        nc.gpsimd.dma_start(out=blob_sb[:, O_XQ:O_XQ + T],
                            in_=blob[:, O_XQ:O_XQ + T])

        xqT = blob_sb[:, O_XQ:O_XQ + T]
        xkvT = blob_sb[:, O_XKV:O_XKV + T]
        wq_sb = blob_sb[:, O_WQ:O_WQ + C]
        wkv_sb = blob_sb[:, O_WKV:O_WKV + 2 * C]
        wp_sb = blob_sb[:, O_WP:O_WP + C]
        srw3 = blob_sb[:, O_SRW:O_SRW + 16 * C].rearrange("c (t o) -> c t o", t=16)
        ones128 = blob_sb[:, O_ONES:O_ONES + C]
        ones_col = ones128[:, 0:1]
        inv128_col = blob_sb[:, O_INV:O_INV + 1]
        ones_row = ones128[0:1, :]
        ones64 = ones128[:, 0:64]
        vecs_f = blob_sb[:, O_VECS:O_VECS + 8].bitcast(F32)
        srb_sb = vecs_f[:, 0:1]
        lnw_sb = vecs_f[:, 1:2]
        lnb_sb = vecs_f[:, 2:3]
        eps_t = vecs_f[0:1, 3:4]

        # dummy engine reads of the blob: advance DVE/ACT observed clocks
        # past the load DMA so scalar-pointer ops need no extra wait
        vtouch = const.tile([1, 1], F32)
        nc.vector.tensor_copy(out=vtouch, in_=vecs_f[0:1, 0:1])
        vtouch2 = const.tile([1, 1], F32)
        nc.scalar.copy(out=vtouch2, in_=vecs_f[0:1, 0:1])

        acts = tc.alloc_tile_pool(name="acts", bufs=1)
        qT = acts.tile([128, T], BF16)     # [qchan, t], pre-scaled by 1/sqrt(hd)
        lnT = acts.tile([128, NK], BF16)   # [c, pos]
        kT = acts.tile([128, NK], BF16)    # [(h d), pos]
        v_sb = acts.tile([128, 2, 128], BF16)  # [pos-part, jt, (h d)]

        # tiny PE matmul reading xkvT: advances PE's observed clock past
        # the xkv load DMA so the first conv matmul (whose psum slot also
        # carries a ghost dependency) needs only one wait
        xt_ps = psum.tile([1, 1], F32, tag="big", bufs=2)
        nc.tensor.matmul(xt_ps, lhsT=xkvT[0:1, 0:1], rhs=xkvT[0:1, 0:1],
                         start=True, stop=True)

        prev_otg = None
        it_dummy = None
        for its in range(niter):
            do_store = (not store_last_only) or (its == niter - 1)
            if its > 0 and prev_otg is not None:
                it_ps = psum.tile([1, 1], F32, tag="big", bufs=2,
                                  name=f"it_ps_{its}")
                it_dummy = nc.tensor.matmul(
                    it_ps, lhsT=prev_otg[0:1, 7, 255:256],
                    rhs=prev_otg[0:1, 7, 255:256],
                    start=True, stop=True)
            # ---------------- conv (spatial reduction) ----------------
            # psum[o, (oh ow)] += srw_tap[i, o].T @ x[i, (4oh+kh)*64 + 4ow+kw]
            x5 = xkvT.rearrange("c (oh kh ow kw) -> c oh kh ow kw", oh=OH, kh=4, ow=OW, kw=4)
            conv_ps = psum.tile([128, NK], F32, tag="b512", bufs=2)
            # two output-row halves: the first 8 oh-rows only need the first
            # half of xkv, so conv starts as soon as that DMA half lands
            for ohh in range(2):
                for tap in range(16):
                    kh, kw = tap // 4, tap % 4
                    nc.tensor.matmul(
                        conv_ps[:, ohh * 128:(ohh + 1) * 128],
                        lhsT=srw3[:, tap, :],
                        rhs=x5[:, ohh * 8:(ohh + 1) * 8, kh, :, kw],
                        start=(tap == 0), stop=(tap == 15))

            # ---------------- LayerNorm over channels (partitions) ----------
            convT = work.tile([128, NK], BF16, tag="convT")
            nc.vector.tensor_scalar_add(out=convT, in0=conv_ps, scalar1=srb_sb)
            ctouch = work.tile([1, 1], BF16, tag="ctouch")
            nc.scalar.copy(out=ctouch, in_=convT[0:1, 0:1])
            sq = work.tile([128, NK], BF16, tag="sq")
            nc.scalar.activation(out=sq, in_=conv_ps,
                                 func=mybir.ActivationFunctionType.Square,
                                 bias=srb_sb, scale=1.0)
            st_obs = None
            if its > 0:
                cdmy = psum.tile([1, 1], F32, tag="big", bufs=2,
                                 name=f"i{its}_cdmy")
                st_obs = nc.tensor.matmul(cdmy, lhsT=convT[0:1, 0:1],
                                          rhs=convT[0:1, 0:1],
                                          start=True, stop=True)
            st_ps = psum.tile([1, 2 * NK], F32, tag="misc", bufs=2)
            smi = nc.tensor.matmul(st_ps[:, 0:NK], lhsT=inv128_col, rhs=convT,
                                   start=True, stop=True)
            if st_obs is not None:
                add_dep_helper(smi.ins, st_obs.ins, sync=False,
                               reason="stats after conv observer")
            nc.tensor.matmul(st_ps[:, NK:2 * NK], lhsT=inv128_col, rhs=sq,
                             start=True, stop=True)
            # st_ps holds mu | E[x^2] (ones column pre-scaled by 1/128)

            brow = work.tile([1, 2 * NK], BF16, tag="brow")
            nc.scalar.mul(out=brow[:, 0:NK], in_=st_ps[:, 0:NK], mul=1.0)
            mu2 = work.tile([1, NK], F32, tag="mu2")
            nc.scalar.square(out=mu2, in_=st_ps[:, 0:NK])
            ex2 = work.tile([1, NK], F32, tag="ex2")
            nc.scalar.mul(out=ex2, in_=st_ps[:, NK:2 * NK], mul=1.0)
            var = work.tile([1, NK], F32, tag="var")
            nc.vector.tensor_sub(out=var, in0=ex2, in1=mu2)
            # rstd = exp(-0.5 * ln(var + eps)); Ln+Exp share one ACT table set
            lnv = work.tile([1, NK], F32, tag="lnv")
            nc.scalar.activation(out=lnv, in_=var,
                                 func=mybir.ActivationFunctionType.Ln,
                                 bias=eps_t, scale=1.0)
            nc.scalar.activation(out=brow[:, NK:2 * NK], in_=lnv,
                                 func=mybir.ActivationFunctionType.Exp, scale=-0.5)
            bc_ps = psum.tile([128, 2 * NK], F32, tag="misc", bufs=2)
            nc.tensor.matmul(bc_ps, lhsT=ones_row, rhs=brow, start=True, stop=True)

            btouch = work.tile([1, 1], F32, tag="btouch")
            nc.vector.tensor_copy(out=btouch, in_=bc_ps[0:1, 0:1])
            t1 = work.tile([128, NK], BF16, tag="t1")
            nc.vector.tensor_sub(out=t1, in0=convT, in1=bc_ps[:, 0:NK])
            t2 = work.tile([128, NK], BF16, tag="t2")
            nc.vector.tensor_mul(out=t2, in0=t1, in1=bc_ps[:, NK:2 * NK])
            nc.vector.tensor_scalar(out=lnT, in0=t2, scalar1=lnw_sb, scalar2=lnb_sb,
                                    op0=mybir.AluOpType.mult, op1=mybir.AluOpType.add)

            # ---------------- k / v projections ----------------
            kv_ps = psum.tile([128, NK], F32, tag="misc", bufs=2)
            nc.tensor.matmul(kv_ps, lhsT=wkv_sb[:, 0:C], rhs=lnT,
                             start=True, stop=True)
            nc.vector.tensor_copy(out=kT, in_=kv_ps)
            for jt in range(2):
                v_ps = psum.tile([128, 128], F32, tag="misc", bufs=2,
                                 name=f"i{its}_v_ps_{jt}")
                nc.tensor.matmul(v_ps, lhsT=lnT[:, jt * 128:(jt + 1) * 128],
                                 rhs=wkv_sb[:, C:2 * C], start=True, stop=True)
                nc.vector.tensor_copy(out=v_sb[:, jt, :], in_=v_ps)

            # ---------------- q projection ----------------
            for i in range(T // 512):
                qp = psum.tile([128, 512], F32, tag="b512", bufs=2, name=f"i{its}_qp_{i}")
                qmi = nc.tensor.matmul(qp, lhsT=wq_sb, rhs=xqT[:, i * 512:(i + 1) * 512],
                                       start=True, stop=True)
                if it_dummy is not None:
                    add_dep_helper(qmi.ins, it_dummy.ins, sync=False,
                                   reason="qp after iteration-boundary observer")
                nc.vector.tensor_copy(out=qT[:, i * 512:(i + 1) * 512], in_=qp)

            # ------- attention + projection (granule-pipelined) -------
            # Emission order interleaves half-granules of adjacent chunks:
            #   a0(sc) scores+exp half0 | a1(sc) half1 | b0/b1(sc) av+dn+div
            #   | d(sc) projection+store
            # ordered so ACT's exp stream never waits on PE: the next
            # chunk's score matmuls are issued inside the current chunk's
            # division window. Tiny observer matmuls keep every
            # instruction within the 1-sync-wait budget.
            # out DRAM view: token = g*2048 + s*256 + 2m + par
            out9 = out[:].rearrange("(g s m j) c -> g m s (j c)", s=8, m=128, j=2)
            allpts = {}
            outTs_by_sc = {}
            avdn_by_sc = {}
            prev_pj_box = [None]
            if its == 0:
                last_otg_box = [None]
                otg_box = [None]
                prev_pts_box = [None]

            def emit_scores(sc, half):
                q0 = sc * SC
                pts = allpts.setdefault(sc, {})
                for jt in range(2):
                    # gate = the exp whose psum slot this tile recycles
                    gate = None
                    if half == 1:
                        gate = pts[jt, 0]
                    elif prev_pts_box[0] is not None:
                        gate = prev_pts_box[0][jt, 1]
                    obs_i = None
                    if gate is not None:
                        dmy = psum.tile([1, 1], F32, tag="b512", bufs=2,
                                        name=f"i{its}_dmyA_{sc}_{half}_{jt}")
                        obs_i = nc.tensor.matmul(dmy, lhsT=gate[0:1, 0:1],
                                                 rhs=gate[0:1, 0:1],
                                                 start=True, stop=True)
                    sps = psum.tile([128, 1024], F32, tag="big", bufs=2,
                                    name=f"i{its}_s_{sc}_{half}_{jt}")
                    for hh in range(2):
                        mi = nc.tensor.matmul(
                            sps[:, hh * 512:(hh + 1) * 512],
                            lhsT=kT[hh * 64:(hh + 1) * 64,
                                    jt * 128:(jt + 1) * 128],
                            rhs=qT[hh * 64:(hh + 1) * 64,
                                   q0 + half * 512:q0 + (half + 1) * 512],
                            start=True, stop=True,
                        )
                        if obs_i is not None:
                            add_dep_helper(mi.ins, obs_i.ins, sync=False,
                                           reason="scores after slot observer")
                    pt = work.tile([128, 1024], BF16, tag="pt", bufs=16,
                                   name=f"i{its}_pt_{sc}_{half}_{jt}")
                    nc.scalar.activation(out=pt, in_=sps,
                                         func=mybir.ActivationFunctionType.Exp)
                    pts[jt, half] = pt
                if half == 1:
                    prev_pts_box[0] = pts

            def emit_avdn(sc, half, tail=False):
                pts = allpts[sc]
                obs2_i = None
                if tail:
                    # last chunk: no next-chunk score observer covers the
                    # half-1 exps, absorb them here
                    dmy4 = psum.tile([1, 1], F32, tag="b512", bufs=2,
                                     name=f"i{its}_dmy4_{sc}")
                    d4 = nc.tensor.matmul(dmy4, lhsT=pts[1, 1][0:1, 0:1],
                                          rhs=pts[1, 1][0:1, 0:1],
                                          start=True, stop=True)
                if half == 1:
                    # DVE observer: half-0 normalize wrote the slot this
                    # av/dn pair recycles
                    dmy3 = psum.tile([1, 1], F32, tag="b512", bufs=2,
                                     name=f"i{its}_dmy3_{sc}")
                    obs2_i = nc.tensor.matmul(
                        dmy3, lhsT=outTs_by_sc[sc][0][0:1, 0:1],
                        rhs=outTs_by_sc[sc][0][0:1, 0:1],
                        start=True, stop=True)
                    if tail:
                        add_dep_helper(obs2_i.ins, d4.ins, sync=False,
                                       reason="dmy3 after tail observer")
                av = psum.tile([128, 512], F32, tag="misc", bufs=2,
                               name=f"i{its}_av_{sc}_{half}")
                dn = psum.tile([128, 512], F32, tag="misc", bufs=2,
                               name=f"i{its}_dn_{sc}_{half}")
                # each head's accumulation group closes before the other
                # opens (CoreSim group tracker); av and dn phase-shifted so
                # opposite PE column groups stay concurrently busy
                def _rhs(hh, jt):
                    return pts[jt, half][:, hh * 512:(hh + 1) * 512]
                for idx in range(2):
                    ah, dh = idx, 1 - idx
                    for jt in range(2):
                        ai = nc.tensor.matmul(
                            av[ah * 64:(ah + 1) * 64, :],
                            lhsT=v_sb[:, jt, ah * 64:(ah + 1) * 64],
                            rhs=_rhs(ah, jt),
                            start=(jt == 0), stop=(jt == 1),
                            tile_position=(0, ah * 64),
                        )
                        if obs2_i is not None:
                            add_dep_helper(ai.ins, obs2_i.ins, sync=False,
                                           reason="av after DVE observer")
                        if prev_pj_box[0] is not None:
                            add_dep_helper(ai.ins, prev_pj_box[0].ins,
                                           sync=False,
                                           reason="keep av after prev pj")
                        nc.tensor.matmul(
                            dn[dh * 64:(dh + 1) * 64, :],
                            lhsT=ones64,
                            rhs=_rhs(dh, jt),
                            start=(jt == 0), stop=(jt == 1),
                            tile_position=(0, dh * 64),
                        )
                rec = work.tile([128, 512], F32, tag="rec", bufs=8,
                                name=f"i{its}_rec_{sc}_{half}")
                nc.vector.reciprocal(out=rec, in_=dn)
                outT = work.tile([128, 512], BF16, tag="outT", bufs=8,
                                 name=f"i{its}_outT_{sc}_{half}")
                nc.vector.tensor_mul(out=outT, in0=av, in1=rec)
                outTs_by_sc.setdefault(sc, []).append(outT)

            def emit_proj(sc):
                if sc % 2 == 0:
                    otg_box[0] = work.tile([128, 8, 256], out_dt, tag="osb",
                                           bufs=2, name=f"i{its}_otg_{sc // 2}")
                otg = otg_box[0]
                for half in range(2):
                    # t-tiles pick even/odd tokens so stores get
                    # 512B-contiguous (token-pair, channel) runs
                    oT3 = outTs_by_sc[sc][half].rearrange(
                        "c (h m j) -> c h j m", h=2, j=2)
                    for h256 in range(2):
                        for par in range(2):
                            pj = psum.tile(
                                [128, 128], F32, tag="b512", bufs=2,
                                name=f"i{its}_pj_{sc}_{half}_{h256}_{par}")
                            prev_pj_box[0] = nc.tensor.matmul(
                                pj, lhsT=oT3[:, h256, par, :],
                                rhs=wp_sb, start=True, stop=True)
                            s = (sc % 2) * 4 + half * 2 + h256
                            nc.vector.tensor_copy(
                                out=otg[:, s, par * 128:(par + 1) * 128],
                                in_=pj)
                            last_otg_box[0] = (otg, s)
                if sc % 2 == 1 and do_store:
                    nc.sync.dma_start(out=out9[sc // 2], in_=otg)

            for sc in range(NSC):
                emit_scores(sc, 0)
                if sc == 0:
                    emit_scores(0, 1)
                    emit_avdn(0, 0)
                else:
                    emit_avdn(sc - 1, 1)
                    emit_scores(sc, 1)
                    emit_proj(sc - 1)
                    emit_avdn(sc, 0)
            emit_avdn(NSC - 1, 1, tail=True)
            emit_proj(NSC - 1)

        psum.release()
        acts.release()
        work.release()
        const.release()
    return nc


_NC_CACHE = {}


def _get_nc(out_bf16=True):
    key = out_bf16
    if key not in _NC_CACHE:
        _NC_CACHE[key] = build_nc(out_bf16)
    return _NC_CACHE[key]


def make_in_maps(x, Wq, Wkv, sr_w, sr_b, ln_w, ln_b, Wp):
    bf = ml_dtypes.bfloat16
    x = np.asarray(x, np.float32)
    wq_t = (np.asarray(Wq, np.float32).T * SCALE).astype(bf)
    wkv_t = np.asarray(Wkv, np.float32).T.astype(bf)
    wp_t = np.asarray(Wp, np.float32).T.astype(bf)
    srw = np.asarray(sr_w, np.float32).transpose(1, 2, 3, 0).reshape(C, 16 * C).astype(bf)
    vecs = np.stack([np.asarray(sr_b, np.float32),
                     np.asarray(ln_w, np.float32),
                     np.asarray(ln_b, np.float32),
                     np.full(C, 1e-5, np.float32)], axis=1)
    vecs_bits = np.ascontiguousarray(vecs).view(np.uint16).view(bf)

    wpart = np.empty((C, NBLOB - O_WQ), bf)
    wpart[:, 0:C] = wq_t
    wpart[:, O_WKV - O_WQ:O_WP - O_WQ] = wkv_t
    wpart[:, O_WP - O_WQ:O_SRW - O_WQ] = wp_t
    wpart[:, O_SRW - O_WQ:O_ONES - O_WQ] = srw
    wpart[:, O_ONES - O_WQ:O_INV - O_WQ] = np.ones((C, C), bf)
    inv = np.zeros((C, 8), np.float32)
    inv[:, 0] = 1.0 / 128.0
    wpart[:, O_INV - O_WQ:O_VECS - O_WQ] = inv.astype(bf)
    wpart[:, O_VECS - O_WQ:] = vecs_bits

    xT = np.ascontiguousarray(x.transpose(0, 2, 1)).astype(bf)  # [B, C, T]
    in_maps = []
    for i in range(8):
        p = (i + 4) % 8
        blob = np.empty((C, NBLOB), bf)
        blob[:, O_XQ:O_XQ + T] = xT[i]
        blob[:, O_XKV:O_XKV + T] = xT[p]
        blob[:, O_WQ:] = wpart
        in_maps.append({"blob": blob})
    return in_maps


def kernel(x, Wq, Wkv, sr_w, sr_b, ln_w, ln_b, Wp, bp, H, W):
    assert int(H) == 64 and int(W) == 64
    in_maps = make_in_maps(x, Wq, Wkv, sr_w, sr_b, ln_w, ln_b, Wp)
    nc = _get_nc(out_bf16=True)
    res = run_bass_kernel_spmd(nc, in_maps, list(range(8)))
    outs = res.results
    r = np.stack([np.asarray(outs[i]["out"], np.float32) for i in range(8)])
    return r + np.asarray(bp, np.float32)[None, None, :]

